# revision 16
# baseline (speedup 1.0000x reference)
"""Low-rank (LoRA) linear for Trainium2, 8 NeuronCores.

Reference math:  out = x @ W^T + b + (ALPHA/R) * (x @ A^T) @ B^T
  x: (4, 2048, 4096) f32, W: (4096, 4096), b: (4096,), A: (16, 4096), B: (4096, 16)

Strategy:
  * Fold the adapter on the host: W_eff = W + SCALE * (B @ A).  The kernel is
    then a single dense GEMM  out = x @ W_eff^T  (+ bias added on host).
  * Data-parallel over tokens: 8192 tokens -> 8 cores x 1024 tokens.
  * Mixed precision: leading 2560 of K in bf16, trailing 1536 of K in
    fp8e4 with perf_mode=DoubleRow (2 fp8 weights per PE cell -> 2 K-rows
    per pass, ~2x matmul rate).  Predicted end-to-end rel err 1.96e-2
    (budget 2e-2), validated against a bit-accurate numpy model of the
    TRN quantization (model matched HW to 5e-6 abs on two configs).
    W is pre-scaled by 16 on the host so W_eff lands in fp8e4's normal
    range; the eviction multiplies PSUM by 1/16 (DVE tensor_scalar_mul,
    same cost as the copy it replaces).
  * Startup: loads stream down three DMA rings (sync + scalar HWDGE,
    gpsimd SWDGE) in exact consumption order, fine-grained (block 0 W in
    256 KB pieces).  Block 0 runs piece-major across 8 concurrent PSUM
    banks so every arriving W piece unlocks work for all 8 token tiles;
    blocks 1-7 run st-major.  Dummy warm-up matmuls hold the PE HAM
    clock-gate at 8/8 while the first chunks arrive.
  * Stores (bf16) go down the scalar ring (free after startup).
    f32 cast + bias on host.
"""

import os

os.environ.setdefault("MYCRO_LOCAL_CACHE", "1")

import numpy as np
import ml_dtypes

R = 16
ALPHA = 32.0
SCALE = ALPHA / R

P = 128          # partitions
D = 4096         # d_in (contraction)
O = 4096         # d_out
S_FULL = 8192    # 4*2048 tokens
N_CORES = 8
S = S_FULL // N_CORES   # tokens per core
DO = D // P             # 32 contraction chunks of 128
ST = S // P             # 8 token tiles per core
NB = 512                # output cols per matmul (one PSUM bank, f32)
OE = O // NB            # 8 output-column blocks

FP8_DOS = 12            # trailing do-chunks (128 K each) in fp8 DoubleRow
BF_DOS = DO - FP8_DOS   # leading do-chunks in bf16 = 22
# bf16 W chunks per o-block: 4-do chunks + remainder
W_CH = [4] * (BF_DOS // 4) + ([BF_DOS % 4] if BF_DOS % 4 else [])   # [4]*5+[2]
# fp8 W chunks per o-block (whole DoubleRow pairs each)
W8_CH = [4, 6] if FP8_DOS == 10 else [4] * (FP8_DOS // 4)
# bf16 x chunks per token tile: 4-do chunks + remainder (fine-grained so
# startup delivery interleaves with W pieces)
X_CH = [4] * (BF_DOS // 4) + ([BF_DOS % 4] if BF_DOS % 4 else [])   # [4]*5+[2]
W_BUFS = 26             # bf16 W chunk slots
W8_BUFS = 12            # fp8 W chunk slots
N_WARM = 12             # PE warm-up matmuls
WSC = 16.0              # host W pre-scale (power of two; undone at evict)

BF16 = ml_dtypes.bfloat16
FP8 = ml_dtypes.float8_e4m3

_cache = {}


def _csum(lst):
    o, s = [], 0
    for v in lst:
        o.append(s)
        s += v
    return o


W_OFF = _csum(W_CH)
W8_OFF = _csum(W8_CH)
X_OFF = _csum(X_CH)
# do -> (x chunk index, index within chunk)
_XQ_OF = []
for _q, _n in enumerate(X_CH):
    for _k in range(_n):
        _XQ_OF.append((_q, _k))


def _build_module():
    import concourse.mybir as mybir
    import concourse.tile as tile
    from concourse import bacc

    DR = mybir.MatmulPerfMode.DoubleRow

    nc = bacc.Bacc(
        "TRN2", target_bir_lowering=False, debug=False, num_devices=N_CORES
    )
    xT = nc.dram_tensor(
        "xT", (ST, P, BF_DOS, P), mybir.dt.bfloat16, kind="ExternalInput"
    ).ap()
    xT8 = nc.dram_tensor(
        "xT8", (ST, P, FP8_DOS, P), mybir.dt.float8e4, kind="ExternalInput"
    ).ap()
    wT = nc.dram_tensor(
        "wT", (OE, P, BF_DOS, NB), mybir.dt.bfloat16, kind="ExternalInput"
    ).ap()
    wT8 = nc.dram_tensor(
        "wT8", (OE, P, FP8_DOS, NB), mybir.dt.float8e4, kind="ExternalInput"
    ).ap()
    out = nc.dram_tensor("out", (S, O), mybir.dt.bfloat16, kind="ExternalOutput").ap()

    with tile.TileContext(nc) as tc:
        with tc.tile_pool(name="xp", bufs=1) as xp, \
             tc.tile_pool(name="wp", bufs=W_BUFS) as wp, \
             tc.tile_pool(name="w8p", bufs=W8_BUFS) as w8p, \
             tc.tile_pool(name="zp", bufs=1) as zp, \
             tc.tile_pool(name="op", bufs=4) as op, \
             tc.tile_pool(name="pp", bufs=8, space="PSUM") as pp:

            # ---- PE warm-up: junk matmuls with no DMA dependency so the
            # HAM clock-gate reaches 8/8 while the first chunks stream in.
            wz = zp.tile([P, NB], mybir.dt.bfloat16)
            nc.vector.memset(wz[:], 0.0)
            wps = pp.tile([P, NB], mybir.dt.float32, tag="ps")
            for _ in range(N_WARM):
                nc.tensor.matmul(
                    wps[:], wz[:, :P], wz[:], start=True, stop=True
                )

            # ---- SBUF tiles
            x_c = [[xp.tile([P, n, P], mybir.dt.bfloat16,
                            tag=f"x{st}_{q}", name=f"x{st}_{q}")
                    for q, n in enumerate(X_CH)] for st in range(ST)]
            x_8 = [xp.tile([P, FP8_DOS, P], mybir.dt.float8e4,
                           tag=f"x8_{st}", name=f"x8_{st}")
                   for st in range(ST)]
            w_c = {}
            w_8 = {}

            # Loads rotate over three rings in consumption order; the SDMA
            # engines round-robin rings at packet granularity, so this
            # approximates one ordered stream at full HBM bandwidth.
            rings = [nc.sync, nc.scalar, nc.gpsimd]
            ring_i = [0]

            def ring():
                r = rings[ring_i[0] % 3]
                ring_i[0] += 1
                return r

            def push_w(oe, c, half=None):
                """bf16 W chunk c; half=0/1 pushes 2-do pieces of a 4-do
                chunk (block-0 startup granularity)."""
                n = W_CH[c]
                if half is None:
                    t = wp.tile([P, n, NB], mybir.dt.bfloat16, tag="w",
                                name=f"w{oe}_{c}")
                    ring().dma_start(
                        out=t[:], in_=wT[oe, :, W_OFF[c]:W_OFF[c] + n, :]
                    )
                    w_c[(oe, c)] = t
                else:
                    o = W_OFF[c] + half * 2
                    t = wp.tile([P, 2, NB], mybir.dt.bfloat16, tag="w",
                                name=f"w{oe}_{c}_{half}")
                    ring().dma_start(out=t[:], in_=wT[oe, :, o:o + 2, :])
                    w_c[(oe, c, half)] = t

            def push_w8(oe, h):
                n = W8_CH[h]
                t = w8p.tile([P, n, NB], mybir.dt.float8e4, tag="w8",
                             name=f"w8_{oe}_{h}")
                ring().dma_start(
                    out=t[:], in_=wT8[oe, :, W8_OFF[h]:W8_OFF[h] + n, :]
                )
                w_8[(oe, h)] = t

            def push_x(st, q):
                ring().dma_start(
                    out=x_c[st][q][:],
                    in_=xT[st, :, X_OFF[q]:X_OFF[q] + X_CH[q], :],
                )

            def push_x8(st):
                ring().dma_start(out=x_8[st][:], in_=xT8[st])

            # ---- startup loads in exact consumption order.
            # Super-block A = (oe 0-1, st 0-3) piece-major: each 2-do W
            # piece feeds 8 groups, so early demand stays under the DMA
            # ramp rate.  B = (oe 0-1, st 4-7) reuses the resident W.
            pieces = []
            for c in range(len(W_CH)):
                for half in range(W_CH[c] // 2):
                    pieces.append((c, half) if W_CH[c] == 4 else (c, None))

            x_pushed = set()
            for pi, (c, half) in enumerate(pieces):
                push_w(0, c, half)
                push_w(1, c, half)
                lo = W_OFF[c] + (half or 0) * 2
                for do in (lo, lo + 1):
                    q = _XQ_OF[do][0]
                    if q not in x_pushed:
                        x_pushed.add(q)
                        for st in range(4):
                            push_x(st, q)
            for h in range(len(W8_CH)):
                push_w8(0, h)
                push_w8(1, h)
                if h == 0:
                    for st in range(4):
                        push_x8(st)
            # B-pass data + o-block 2 prefetch
            for q in range(len(X_CH)):
                for st in range(4, ST):
                    push_x(st, q)
            for st in range(4, ST):
                push_x8(st)
            for c in range(len(W_CH)):
                push_w(2, c)
            for h in range(len(W8_CH)):
                push_w8(2, h)

            def evict(oe, st, ps_t):
                o_sb = op.tile([P, NB], mybir.dt.bfloat16, tag="o",
                               name=f"o{oe}_{st}")
                nc.vector.tensor_scalar_mul(o_sb[:], ps_t[:], 1.0 / WSC)
                nc.scalar.dma_start(
                    out=out[st * P:(st + 1) * P, oe * NB:(oe + 1) * NB],
                    in_=o_sb[:],
                )

            def mm_bf(ps_t, st, do, wt, wdo, start):
                q, k = _XQ_OF[do]
                nc.tensor.matmul(
                    ps_t[:],
                    x_c[st][q][:, k, :],
                    wt[:, wdo, :],
                    start=start, stop=False,
                )

            def mm_dr(ps_t, st, j, wt, wj, stop):
                """DoubleRow pair j (fp8 dos 2j, 2j+1)."""
                nc.tensor.matmul(
                    ps_t[:],
                    x_8[st][:, 2 * j:2 * j + 2, :],
                    wt[:, 2 * wj:2 * wj + 2, :],
                    start=False, stop=stop,
                    perf_mode=DR,
                )

            n_pairs = FP8_DOS // 2
            h_of = []                       # pair j -> (chunk h, local pair)
            for h, n in enumerate(W8_CH):
                for lp in range(n // 2):
                    h_of.append((h, lp))

            # ---- super-blocks A (st 0-3) and B (st 4-7) over oe 0-1,
            # piece-major with 8 open PSUM groups (2 oe x 4 st).
            for half_pass, sts in ((0, range(4)), (1, range(4, ST))):
                ps = {(oe, st): pp.tile([P, NB], mybir.dt.float32,
                                        tag="ps", name=f"ps{oe}_{st}")
                      for oe in range(2) for st in sts}
                for pi, (c, half) in enumerate(pieces):
                    base = W_OFF[c] + (half or 0) * 2
                    for oe in range(2):
                        wt = w_c[(0 + oe, c, half)]
                        for st in sts:
                            for i in range(2):
                                do = base + i
                                mm_bf(ps[(oe, st)], st, do, wt, i,
                                      start=(do == 0))
                    if half_pass == 1 and pi == 4:
                        # o-block 3 prefetch mid-B (slots free by now)
                        for c3 in range(len(W_CH)):
                            push_w(3, c3)
                        for h3 in range(len(W8_CH)):
                            push_w8(3, h3)
                for h in range(len(W8_CH)):
                    for oe in range(2):
                        wt = w_8[(oe, h)]
                        for st in sts:
                            for lp in range(W8_CH[h] // 2):
                                j = W8_OFF[h] // 2 + lp
                                mm_dr(ps[(oe, st)], st, j, wt, lp,
                                      stop=(j == n_pairs - 1))
                for oe in range(2):
                    for st in sts:
                        evict(oe, st, ps[(oe, st)])

            # ---- o-blocks 2..7: st-major; evictions pipeline.
            for oe in range(2, OE):
                wts = [w_c.pop((oe, c)) for c in range(len(W_CH))]
                w8ts = [w_8.pop((oe, h)) for h in range(len(W8_CH))]
                for st in range(ST):
                    if st == 4 and oe + 2 < OE:
                        # prefetch W two blocks out, mid-block so the
                        # push never waits on a busy slot (no ring stall)
                        for c2 in range(len(W_CH)):
                            push_w(oe + 2, c2)
                        for h2 in range(len(W8_CH)):
                            push_w8(oe + 2, h2)
                    ps_t = pp.tile([P, NB], mybir.dt.float32, tag="ps",
                                   name=f"ps{oe}_{st}")
                    for do in range(BF_DOS):
                        c = min(do // 4, len(W_CH) - 1)
                        mm_bf(ps_t, st, do, wts[c], do - W_OFF[c],
                              start=(do == 0))
                    for j in range(n_pairs):
                        h, lp = h_of[j]
                        mm_dr(ps_t, st, j, w8ts[h], lp,
                              stop=(j == n_pairs - 1))
                    evict(oe, st, ps_t)
    nc.compile()
    return nc


def _get_module():
    if "nc" not in _cache:
        _cache["nc"] = _build_module()
    return _cache["nc"]


def _prep_inputs(x, W, b, A, B):
    """Host-side: fold adapter, transpose to kernel layouts, cast, shard."""
    W_eff = W.astype(np.float32) + SCALE * (
        B.astype(np.float32) @ A.astype(np.float32)
    )
    W16 = W_eff * WSC
    # wT[oe, p, do, oo] = W16[oe*NB+oo, do*P+p]  (= W16^T in [K,N] tiles)
    wfull = np.ascontiguousarray(
        W16.T.reshape(DO, P, OE, NB).transpose(2, 1, 0, 3)
    )
    wT = np.ascontiguousarray(wfull[:, :, :BF_DOS, :]).astype(BF16)
    wT8 = np.ascontiguousarray(wfull[:, :, BF_DOS:, :]).astype(FP8)
    x2 = np.asarray(x, dtype=np.float32).reshape(S_FULL, D)
    in_maps = []
    for c in range(N_CORES):
        xc = x2[c * S:(c + 1) * S]                       # (S, D)
        # xT[st, p, do, s'] = xc[st*P+s', do*P+p]  (contiguous per (st, p))
        xfull = np.ascontiguousarray(
            xc.reshape(ST, P, DO, P).transpose(0, 3, 2, 1)
        )
        xTc = np.ascontiguousarray(xfull[:, :, :BF_DOS, :]).astype(BF16)
        xT8c = np.ascontiguousarray(xfull[:, :, BF_DOS:, :]).astype(FP8)
        in_maps.append({"xT": xTc, "xT8": xT8c, "wT": wT, "wT8": wT8})
    return in_maps


def run(x, W, b, A, B, trace=False, **spmd_kwargs):
    """Run the kernel; returns (full_output, BassKernelResults)."""
    from concourse import bass_utils

    nc = _get_module()
    in_maps = _prep_inputs(x, W, b, A, B)
    res = bass_utils.run_bass_kernel_spmd(
        nc, in_maps, core_ids=list(range(N_CORES)), trace=trace, **spmd_kwargs
    )
    outs = [np.asarray(res.results[c]["out"]) for c in range(N_CORES)]
    full = np.concatenate(outs, axis=0).astype(np.float32)
    full += np.asarray(b, dtype=np.float32)
    return full.reshape(4, 2048, O), res


def kernel(x, W, b, A, B):
    full, _ = run(x, W, b, A, B, trace=False)
    return full


# revision 17
# speedup vs baseline: 1.0209x; 1.0209x over previous
"""Low-rank (LoRA) linear for Trainium2, 8 NeuronCores.

Reference math:  out = x @ W^T + b + (ALPHA/R) * (x @ A^T) @ B^T
  x: (4, 2048, 4096) f32, W: (4096, 4096), b: (4096,), A: (16, 4096), B: (4096, 16)

Strategy:
  * Fold the adapter on the host: W_eff = W + SCALE * (B @ A).  The kernel is
    then a single dense GEMM  out = x @ W_eff^T  (+ bias added on host).
  * Data-parallel over tokens: 8192 tokens -> 8 cores x 1024 tokens.
  * Mixed precision: leading 2560 of K in bf16, trailing 1536 of K in
    fp8e4 with perf_mode=DoubleRow (2 fp8 weights per PE cell -> 2 K-rows
    per pass, ~2x matmul rate).  Predicted end-to-end rel err 1.96e-2
    (budget 2e-2), validated against a bit-accurate numpy model of the
    TRN quantization (model matched HW to 5e-6 abs on two configs).
    W is pre-scaled by 16 on the host so W_eff lands in fp8e4's normal
    range; the eviction multiplies PSUM by 1/16 (DVE tensor_scalar_mul,
    same cost as the copy it replaces).
  * Startup: loads stream down two DMA rings (sync HWDGE, gpsimd SWDGE)
    in exact consumption order, fine-grained 256 KB pieces.  The first
    phase is a 2x-oe super-block (o-blocks 0-1 x token tiles 0-3,
    piece-major over 8 PSUM banks) so each arriving W piece feeds 8
    accumulation groups and early DMA demand stays under the ramp rate;
    a second pass covers tokens 4-7 with W resident, then o-blocks 2-7
    run st-major with W prefetched two blocks ahead (pushed mid-block so
    a push never head-of-line blocks its ring).  Dummy warm-up matmuls
    hold the PE HAM clock-gate at 8/8 while the first pieces arrive.
  * Stores (bf16) go down the scalar ring EXCLUSIVELY — prefetch pushes
    on the store ring would head-of-line block evictions and stall PSUM
    recycling.  f32 cast + bias on host.
"""

import os

os.environ.setdefault("MYCRO_LOCAL_CACHE", "1")

import numpy as np
import ml_dtypes

R = 16
ALPHA = 32.0
SCALE = ALPHA / R

P = 128          # partitions
D = 4096         # d_in (contraction)
O = 4096         # d_out
S_FULL = 8192    # 4*2048 tokens
N_CORES = 8
S = S_FULL // N_CORES   # tokens per core
DO = D // P             # 32 contraction chunks of 128
ST = S // P             # 8 token tiles per core
NB = 512                # output cols per matmul (one PSUM bank, f32)
OE = O // NB            # 8 output-column blocks

FP8_DOS = 12            # trailing do-chunks (128 K each) in fp8 DoubleRow
BF_DOS = DO - FP8_DOS   # leading do-chunks in bf16 = 22
# bf16 W chunks per o-block: 4-do chunks + remainder
W_CH = [4] * (BF_DOS // 4) + ([BF_DOS % 4] if BF_DOS % 4 else [])   # [4]*5+[2]
# fp8 W chunks per o-block (whole DoubleRow pairs each)
W8_CH = [4, 6] if FP8_DOS == 10 else [4] * (FP8_DOS // 4)
# bf16 x chunks per token tile: 4-do chunks + remainder (fine-grained so
# startup delivery interleaves with W pieces)
X_CH = [4] * (BF_DOS // 4) + ([BF_DOS % 4] if BF_DOS % 4 else [])   # [4]*5+[2]
W_BUFS = 26             # bf16 W chunk slots
W8_BUFS = 12            # fp8 W chunk slots
N_WARM = 12             # PE warm-up matmuls
WSC = 16.0              # host W pre-scale (power of two; undone at evict)

BF16 = ml_dtypes.bfloat16
FP8 = ml_dtypes.float8_e4m3

_cache = {}


def _csum(lst):
    o, s = [], 0
    for v in lst:
        o.append(s)
        s += v
    return o


W_OFF = _csum(W_CH)
W8_OFF = _csum(W8_CH)
X_OFF = _csum(X_CH)
# do -> (x chunk index, index within chunk)
_XQ_OF = []
for _q, _n in enumerate(X_CH):
    for _k in range(_n):
        _XQ_OF.append((_q, _k))


def _build_module():
    import concourse.mybir as mybir
    import concourse.tile as tile
    from concourse import bacc

    DR = mybir.MatmulPerfMode.DoubleRow

    nc = bacc.Bacc(
        "TRN2", target_bir_lowering=False, debug=False, num_devices=N_CORES
    )
    xT = nc.dram_tensor(
        "xT", (ST, P, BF_DOS, P), mybir.dt.bfloat16, kind="ExternalInput"
    ).ap()
    xT8 = nc.dram_tensor(
        "xT8", (ST, P, FP8_DOS, P), mybir.dt.float8e4, kind="ExternalInput"
    ).ap()
    wT = nc.dram_tensor(
        "wT", (OE, P, BF_DOS, NB), mybir.dt.bfloat16, kind="ExternalInput"
    ).ap()
    wT8 = nc.dram_tensor(
        "wT8", (OE, P, FP8_DOS, NB), mybir.dt.float8e4, kind="ExternalInput"
    ).ap()
    out = nc.dram_tensor("out", (S, O), mybir.dt.bfloat16, kind="ExternalOutput").ap()

    with tile.TileContext(nc) as tc:
        with tc.tile_pool(name="xp", bufs=1) as xp, \
             tc.tile_pool(name="wp", bufs=W_BUFS) as wp, \
             tc.tile_pool(name="w8p", bufs=W8_BUFS) as w8p, \
             tc.tile_pool(name="zp", bufs=1) as zp, \
             tc.tile_pool(name="op", bufs=12) as op, \
             tc.tile_pool(name="pp", bufs=8, space="PSUM") as pp:

            # ---- PE warm-up: junk matmuls with no DMA dependency so the
            # HAM clock-gate reaches 8/8 while the first chunks stream in.
            wz = zp.tile([P, NB], mybir.dt.bfloat16)
            nc.vector.memset(wz[:], 0.0)
            wps = pp.tile([P, NB], mybir.dt.float32, tag="ps")
            for _ in range(N_WARM):
                nc.tensor.matmul(
                    wps[:], wz[:, :P], wz[:], start=True, stop=True
                )

            # ---- SBUF tiles
            x_c = [[xp.tile([P, n, P], mybir.dt.bfloat16,
                            tag=f"x{st}_{q}", name=f"x{st}_{q}")
                    for q, n in enumerate(X_CH)] for st in range(ST)]
            x_8 = [xp.tile([P, FP8_DOS, P], mybir.dt.float8e4,
                           tag=f"x8_{st}", name=f"x8_{st}")
                   for st in range(ST)]
            w_c = {}
            w_8 = {}

            # Loads rotate over three rings in consumption order; the SDMA
            # engines round-robin rings at packet granularity, so this
            # approximates one ordered stream at full HBM bandwidth.
            rings = [nc.sync, nc.gpsimd]
            ring_i = [0]

            def ring():
                r = rings[ring_i[0] % 2]
                ring_i[0] += 1
                return r

            def push_w(oe, c, half=None):
                """bf16 W chunk c; half=0/1 pushes 2-do pieces of a 4-do
                chunk (block-0 startup granularity)."""
                n = W_CH[c]
                if half is None:
                    t = wp.tile([P, n, NB], mybir.dt.bfloat16, tag="w",
                                name=f"w{oe}_{c}")
                    ring().dma_start(
                        out=t[:], in_=wT[oe, :, W_OFF[c]:W_OFF[c] + n, :]
                    )
                    w_c[(oe, c)] = t
                else:
                    o = W_OFF[c] + half * 2
                    t = wp.tile([P, 2, NB], mybir.dt.bfloat16, tag="w",
                                name=f"w{oe}_{c}_{half}")
                    ring().dma_start(out=t[:], in_=wT[oe, :, o:o + 2, :])
                    w_c[(oe, c, half)] = t

            def push_w8(oe, h):
                n = W8_CH[h]
                t = w8p.tile([P, n, NB], mybir.dt.float8e4, tag="w8",
                             name=f"w8_{oe}_{h}")
                ring().dma_start(
                    out=t[:], in_=wT8[oe, :, W8_OFF[h]:W8_OFF[h] + n, :]
                )
                w_8[(oe, h)] = t

            def push_x(st, q):
                ring().dma_start(
                    out=x_c[st][q][:],
                    in_=xT[st, :, X_OFF[q]:X_OFF[q] + X_CH[q], :],
                )

            def push_x8(st):
                ring().dma_start(out=x_8[st][:], in_=xT8[st])

            # ---- startup loads in exact consumption order.
            # Super-block A = (oe 0-1, st 0-3) piece-major: each 2-do W
            # piece feeds 8 groups, so early demand stays under the DMA
            # ramp rate.  B = (oe 0-1, st 4-7) reuses the resident W.
            pieces = []
            for c in range(len(W_CH)):
                for half in range(W_CH[c] // 2):
                    pieces.append((c, half) if W_CH[c] == 4 else (c, None))

            x_pushed = set()
            for pi, (c, half) in enumerate(pieces):
                push_w(0, c, half)
                push_w(1, c, half)
                lo = W_OFF[c] + (half or 0) * 2
                for do in (lo, lo + 1):
                    q = _XQ_OF[do][0]
                    if q not in x_pushed:
                        x_pushed.add(q)
                        for st in range(4):
                            push_x(st, q)
            for h in range(len(W8_CH)):
                push_w8(0, h)
                push_w8(1, h)
                if h == 0:
                    for st in range(4):
                        push_x8(st)
            # B-pass data + o-block 2 prefetch
            for q in range(len(X_CH)):
                for st in range(4, ST):
                    push_x(st, q)
            for st in range(4, ST):
                push_x8(st)
            for c in range(len(W_CH)):
                push_w(2, c)
            for h in range(len(W8_CH)):
                push_w8(2, h)

            def evict(oe, st, ps_t):
                o_sb = op.tile([P, NB], mybir.dt.bfloat16, tag="o",
                               name=f"o{oe}_{st}")
                nc.vector.tensor_scalar_mul(o_sb[:], ps_t[:], 1.0 / WSC)
                nc.scalar.dma_start(
                    out=out[st * P:(st + 1) * P, oe * NB:(oe + 1) * NB],
                    in_=o_sb[:],
                )

            def mm_bf(ps_t, st, do, wt, wdo, start):
                q, k = _XQ_OF[do]
                nc.tensor.matmul(
                    ps_t[:],
                    x_c[st][q][:, k, :],
                    wt[:, wdo, :],
                    start=start, stop=False,
                )

            def mm_dr(ps_t, st, j, wt, wj, stop):
                """DoubleRow pair j (fp8 dos 2j, 2j+1)."""
                nc.tensor.matmul(
                    ps_t[:],
                    x_8[st][:, 2 * j:2 * j + 2, :],
                    wt[:, 2 * wj:2 * wj + 2, :],
                    start=False, stop=stop,
                    perf_mode=DR,
                )

            n_pairs = FP8_DOS // 2
            h_of = []                       # pair j -> (chunk h, local pair)
            for h, n in enumerate(W8_CH):
                for lp in range(n // 2):
                    h_of.append((h, lp))

            # ---- super-blocks A (st 0-3) and B (st 4-7) over oe 0-1,
            # piece-major with 8 open PSUM groups (2 oe x 4 st).
            for half_pass, sts in ((0, range(4)), (1, range(4, ST))):
                ps = {(oe, st): pp.tile([P, NB], mybir.dt.float32,
                                        tag="ps", name=f"ps{oe}_{st}")
                      for oe in range(2) for st in sts}
                for pi, (c, half) in enumerate(pieces):
                    base = W_OFF[c] + (half or 0) * 2
                    for oe in range(2):
                        wt = w_c[(0 + oe, c, half)]
                        for st in sts:
                            for i in range(2):
                                do = base + i
                                mm_bf(ps[(oe, st)], st, do, wt, i,
                                      start=(do == 0))
                    if half_pass == 1 and pi == 4:
                        # o-block 3 prefetch mid-B (slots free by now)
                        for c3 in range(len(W_CH)):
                            push_w(3, c3)
                        for h3 in range(len(W8_CH)):
                            push_w8(3, h3)
                for h in range(len(W8_CH)):
                    for oe in range(2):
                        wt = w_8[(oe, h)]
                        for st in sts:
                            for lp in range(W8_CH[h] // 2):
                                j = W8_OFF[h] // 2 + lp
                                mm_dr(ps[(oe, st)], st, j, wt, lp,
                                      stop=(j == n_pairs - 1))
                for oe in range(2):
                    for st in sts:
                        evict(oe, st, ps[(oe, st)])

            # ---- o-blocks 2..7: st-major; evictions pipeline.
            for oe in range(2, OE):
                wts = [w_c.pop((oe, c)) for c in range(len(W_CH))]
                w8ts = [w_8.pop((oe, h)) for h in range(len(W8_CH))]
                for st in range(ST):
                    if st == 4 and oe + 2 < OE:
                        # prefetch W two blocks out, mid-block so the
                        # push never waits on a busy slot (no ring stall)
                        for c2 in range(len(W_CH)):
                            push_w(oe + 2, c2)
                        for h2 in range(len(W8_CH)):
                            push_w8(oe + 2, h2)
                    ps_t = pp.tile([P, NB], mybir.dt.float32, tag="ps",
                                   name=f"ps{oe}_{st}")
                    for do in range(BF_DOS):
                        c = min(do // 4, len(W_CH) - 1)
                        mm_bf(ps_t, st, do, wts[c], do - W_OFF[c],
                              start=(do == 0))
                    for j in range(n_pairs):
                        h, lp = h_of[j]
                        mm_dr(ps_t, st, j, w8ts[h], lp,
                              stop=(j == n_pairs - 1))
                    evict(oe, st, ps_t)
    nc.compile()
    return nc


def _get_module():
    if "nc" not in _cache:
        _cache["nc"] = _build_module()
    return _cache["nc"]


def _prep_inputs(x, W, b, A, B):
    """Host-side: fold adapter, transpose to kernel layouts, cast, shard."""
    W_eff = W.astype(np.float32) + SCALE * (
        B.astype(np.float32) @ A.astype(np.float32)
    )
    W16 = W_eff * WSC
    # wT[oe, p, do, oo] = W16[oe*NB+oo, do*P+p]  (= W16^T in [K,N] tiles)
    wfull = np.ascontiguousarray(
        W16.T.reshape(DO, P, OE, NB).transpose(2, 1, 0, 3)
    )
    wT = np.ascontiguousarray(wfull[:, :, :BF_DOS, :]).astype(BF16)
    wT8 = np.ascontiguousarray(wfull[:, :, BF_DOS:, :]).astype(FP8)
    x2 = np.asarray(x, dtype=np.float32).reshape(S_FULL, D)
    in_maps = []
    for c in range(N_CORES):
        xc = x2[c * S:(c + 1) * S]                       # (S, D)
        # xT[st, p, do, s'] = xc[st*P+s', do*P+p]  (contiguous per (st, p))
        xfull = np.ascontiguousarray(
            xc.reshape(ST, P, DO, P).transpose(0, 3, 2, 1)
        )
        xTc = np.ascontiguousarray(xfull[:, :, :BF_DOS, :]).astype(BF16)
        xT8c = np.ascontiguousarray(xfull[:, :, BF_DOS:, :]).astype(FP8)
        in_maps.append({"xT": xTc, "xT8": xT8c, "wT": wT, "wT8": wT8})
    return in_maps


def run(x, W, b, A, B, trace=False, **spmd_kwargs):
    """Run the kernel; returns (full_output, BassKernelResults)."""
    from concourse import bass_utils

    nc = _get_module()
    in_maps = _prep_inputs(x, W, b, A, B)
    res = bass_utils.run_bass_kernel_spmd(
        nc, in_maps, core_ids=list(range(N_CORES)), trace=trace, **spmd_kwargs
    )
    outs = [np.asarray(res.results[c]["out"]) for c in range(N_CORES)]
    full = np.concatenate(outs, axis=0).astype(np.float32)
    full += np.asarray(b, dtype=np.float32)
    return full.reshape(4, 2048, O), res


def kernel(x, W, b, A, B):
    full, _ = run(x, W, b, A, B, trace=False)
    return full


# revision 18
# speedup vs baseline: 1.0388x; 1.0175x over previous
"""Low-rank (LoRA) linear for Trainium2, 8 NeuronCores.

Reference math:  out = x @ W^T + b + (ALPHA/R) * (x @ A^T) @ B^T
  x: (4, 2048, 4096) f32, W: (4096, 4096), b: (4096,), A: (16, 4096), B: (4096, 16)

Strategy:
  * Fold the adapter on the host: W_eff = W + SCALE * (B @ A).  The kernel is
    then a single dense GEMM  out = x @ W_eff^T  (+ bias added on host).
  * Data-parallel over tokens: 8192 tokens -> 8 cores x 1024 tokens.
  * Mixed precision: leading 2560 of K in bf16, trailing 1536 of K in
    fp8e4 with perf_mode=DoubleRow (2 fp8 weights per PE cell -> 2 K-rows
    per pass, ~2x matmul rate).  Predicted end-to-end rel err 1.96e-2
    (budget 2e-2), validated against a bit-accurate numpy model of the
    TRN quantization (model matched HW to 5e-6 abs on two configs).
    W is pre-scaled by 16 on the host so W_eff lands in fp8e4's normal
    range; the eviction multiplies PSUM by 1/16 (DVE tensor_scalar_mul,
    same cost as the copy it replaces).
  * Startup: loads stream down two DMA rings (sync HWDGE, gpsimd SWDGE)
    in exact consumption order, fine-grained 256 KB pieces.  The first
    phase is a 2x-oe super-block (o-blocks 0-1 x token tiles 0-3,
    piece-major over 8 PSUM banks) so each arriving W piece feeds 8
    accumulation groups and early DMA demand stays under the ramp rate;
    a second pass covers tokens 4-7 with W resident, then o-blocks 2-7
    run st-major with W prefetched two blocks ahead (pushed mid-block so
    a push never head-of-line blocks its ring).  Dummy warm-up matmuls
    hold the PE HAM clock-gate at 8/8 while the first pieces arrive.
  * Stores (bf16) go down the scalar ring EXCLUSIVELY — prefetch pushes
    on the store ring would head-of-line block evictions and stall PSUM
    recycling.  f32 cast + bias on host.
"""

import os

os.environ.setdefault("MYCRO_LOCAL_CACHE", "1")

import numpy as np
import ml_dtypes

R = 16
ALPHA = 32.0
SCALE = ALPHA / R

P = 128          # partitions
D = 4096         # d_in (contraction)
O = 4096         # d_out
S_FULL = 8192    # 4*2048 tokens
N_CORES = 8
S = S_FULL // N_CORES   # tokens per core
DO = D // P             # 32 contraction chunks of 128
ST = S // P             # 8 token tiles per core
NB = 512                # output cols per matmul (one PSUM bank, f32)
OE = O // NB            # 8 output-column blocks

FP8_DOS = 12            # trailing do-chunks (128 K each) in fp8 DoubleRow
BF_DOS = DO - FP8_DOS   # leading do-chunks in bf16 = 22
# bf16 W chunks per o-block: 4-do chunks + remainder
W_CH = [4] * (BF_DOS // 4) + ([BF_DOS % 4] if BF_DOS % 4 else [])   # [4]*5+[2]
# fp8 W chunks per o-block (whole DoubleRow pairs each)
W8_CH = [4, 6] if FP8_DOS == 10 else [4] * (FP8_DOS // 4)
# bf16 x chunks per token tile: 4-do chunks + remainder (fine-grained so
# startup delivery interleaves with W pieces)
X_CH = [4] * (BF_DOS // 4) + ([BF_DOS % 4] if BF_DOS % 4 else [])   # [4]*5+[2]
W_BUFS = 26             # bf16 W chunk slots
W8_BUFS = 12            # fp8 W chunk slots
N_WARM = 11             # PE warm-up matmuls
WSC = 16.0              # host W pre-scale (power of two; undone at evict)

BF16 = ml_dtypes.bfloat16
FP8 = ml_dtypes.float8_e4m3

_cache = {}


def _csum(lst):
    o, s = [], 0
    for v in lst:
        o.append(s)
        s += v
    return o


W_OFF = _csum(W_CH)
W8_OFF = _csum(W8_CH)
X_OFF = _csum(X_CH)
# do -> (x chunk index, index within chunk)
_XQ_OF = []
for _q, _n in enumerate(X_CH):
    for _k in range(_n):
        _XQ_OF.append((_q, _k))


def _build_module():
    import concourse.mybir as mybir
    import concourse.tile as tile
    from concourse import bacc

    DR = mybir.MatmulPerfMode.DoubleRow

    nc = bacc.Bacc(
        "TRN2", target_bir_lowering=False, debug=False, num_devices=N_CORES
    )
    xT = nc.dram_tensor(
        "xT", (ST, P, BF_DOS, P), mybir.dt.bfloat16, kind="ExternalInput"
    ).ap()
    xT8 = nc.dram_tensor(
        "xT8", (ST, P, FP8_DOS, P), mybir.dt.float8e4, kind="ExternalInput"
    ).ap()
    wT = nc.dram_tensor(
        "wT", (OE, P, BF_DOS, NB), mybir.dt.bfloat16, kind="ExternalInput"
    ).ap()
    wT8 = nc.dram_tensor(
        "wT8", (OE, P, FP8_DOS, NB), mybir.dt.float8e4, kind="ExternalInput"
    ).ap()
    out = nc.dram_tensor("out", (S, O), mybir.dt.bfloat16, kind="ExternalOutput").ap()

    with tile.TileContext(nc) as tc:
        with tc.tile_pool(name="xp", bufs=1) as xp, \
             tc.tile_pool(name="wp", bufs=W_BUFS) as wp, \
             tc.tile_pool(name="w8p", bufs=W8_BUFS) as w8p, \
             tc.tile_pool(name="zp", bufs=1) as zp, \
             tc.tile_pool(name="op", bufs=12) as op, \
             tc.tile_pool(name="pp", bufs=8, space="PSUM") as pp:

            # ---- PE warm-up: junk matmuls with no DMA dependency so the
            # HAM clock-gate reaches 8/8 while the first chunks stream in.
            wz = zp.tile([P, NB], mybir.dt.bfloat16)
            nc.vector.memset(wz[:], 0.0)
            wps = pp.tile([P, NB], mybir.dt.float32, tag="ps")
            for _ in range(N_WARM):
                nc.tensor.matmul(
                    wps[:], wz[:, :P], wz[:], start=True, stop=True
                )

            # ---- SBUF tiles
            x_c = [[xp.tile([P, n, P], mybir.dt.bfloat16,
                            tag=f"x{st}_{q}", name=f"x{st}_{q}")
                    for q, n in enumerate(X_CH)] for st in range(ST)]
            x_8 = [xp.tile([P, FP8_DOS, P], mybir.dt.float8e4,
                           tag=f"x8_{st}", name=f"x8_{st}")
                   for st in range(ST)]
            w_c = {}
            w_8 = {}

            # Loads rotate over three rings in consumption order; the SDMA
            # engines round-robin rings at packet granularity, so this
            # approximates one ordered stream at full HBM bandwidth.
            rings = [nc.sync, nc.gpsimd]
            ring_i = [0]

            def ring():
                r = rings[ring_i[0] % 2]
                ring_i[0] += 1
                return r

            def push_w(oe, c, half=None):
                """bf16 W chunk c; half=0/1 pushes 2-do pieces of a 4-do
                chunk (block-0 startup granularity)."""
                n = W_CH[c]
                if half is None:
                    t = wp.tile([P, n, NB], mybir.dt.bfloat16, tag="w",
                                name=f"w{oe}_{c}")
                    ring().dma_start(
                        out=t[:], in_=wT[oe, :, W_OFF[c]:W_OFF[c] + n, :]
                    )
                    w_c[(oe, c)] = t
                else:
                    o = W_OFF[c] + half * 2
                    t = wp.tile([P, 2, NB], mybir.dt.bfloat16, tag="w",
                                name=f"w{oe}_{c}_{half}")
                    ring().dma_start(out=t[:], in_=wT[oe, :, o:o + 2, :])
                    w_c[(oe, c, half)] = t

            def push_w8(oe, h):
                n = W8_CH[h]
                t = w8p.tile([P, n, NB], mybir.dt.float8e4, tag="w8",
                             name=f"w8_{oe}_{h}")
                ring().dma_start(
                    out=t[:], in_=wT8[oe, :, W8_OFF[h]:W8_OFF[h] + n, :]
                )
                w_8[(oe, h)] = t

            def push_x(st, q):
                ring().dma_start(
                    out=x_c[st][q][:],
                    in_=xT[st, :, X_OFF[q]:X_OFF[q] + X_CH[q], :],
                )

            def push_x8(st):
                ring().dma_start(out=x_8[st][:], in_=xT8[st])

            # ---- startup loads in exact consumption order.
            # Super-block A = (oe 0-1, st 0-3) piece-major: each 2-do W
            # piece feeds 8 groups, so early demand stays under the DMA
            # ramp rate.  B = (oe 0-1, st 4-7) reuses the resident W.
            pieces = []
            for c in range(len(W_CH)):
                for half in range(W_CH[c] // 2):
                    pieces.append((c, half) if W_CH[c] == 4 else (c, None))

            x_pushed = set()
            for pi, (c, half) in enumerate(pieces):
                push_w(0, c, half)
                lo = W_OFF[c] + (half or 0) * 2
                for do in (lo, lo + 1):
                    q = _XQ_OF[do][0]
                    if q not in x_pushed:
                        x_pushed.add(q)
                        for st in range(4):
                            push_x(st, q)
                push_w(1, c, half)
            for h in range(len(W8_CH)):
                push_w8(0, h)
                push_w8(1, h)
                if h == 0:
                    for st in range(4):
                        push_x8(st)
            # B-pass data + o-block 2 prefetch
            for q in range(len(X_CH)):
                for st in range(4, ST):
                    push_x(st, q)
            for st in range(4, ST):
                push_x8(st)
            for c in range(len(W_CH)):
                push_w(2, c)
            for h in range(len(W8_CH)):
                push_w8(2, h)

            def evict(oe, st, ps_t):
                o_sb = op.tile([P, NB], mybir.dt.bfloat16, tag="o",
                               name=f"o{oe}_{st}")
                nc.vector.tensor_scalar_mul(o_sb[:], ps_t[:], 1.0 / WSC)
                nc.scalar.dma_start(
                    out=out[st * P:(st + 1) * P, oe * NB:(oe + 1) * NB],
                    in_=o_sb[:],
                )

            def mm_bf(ps_t, st, do, wt, wdo, start):
                q, k = _XQ_OF[do]
                nc.tensor.matmul(
                    ps_t[:],
                    x_c[st][q][:, k, :],
                    wt[:, wdo, :],
                    start=start, stop=False,
                )

            def mm_dr(ps_t, st, j, wt, wj, stop):
                """DoubleRow pair j (fp8 dos 2j, 2j+1)."""
                nc.tensor.matmul(
                    ps_t[:],
                    x_8[st][:, 2 * j:2 * j + 2, :],
                    wt[:, 2 * wj:2 * wj + 2, :],
                    start=False, stop=stop,
                    perf_mode=DR,
                )

            n_pairs = FP8_DOS // 2
            h_of = []                       # pair j -> (chunk h, local pair)
            for h, n in enumerate(W8_CH):
                for lp in range(n // 2):
                    h_of.append((h, lp))

            # ---- super-blocks A (st 0-3) and B (st 4-7) over oe 0-1,
            # piece-major with 8 open PSUM groups (2 oe x 4 st).
            for half_pass, sts in ((0, range(4)), (1, range(4, ST))):
                ps = {(oe, st): pp.tile([P, NB], mybir.dt.float32,
                                        tag="ps", name=f"ps{oe}_{st}")
                      for oe in range(2) for st in sts}
                for pi, (c, half) in enumerate(pieces):
                    base = W_OFF[c] + (half or 0) * 2
                    for oe in range(2):
                        wt = w_c[(0 + oe, c, half)]
                        for st in sts:
                            for i in range(2):
                                do = base + i
                                mm_bf(ps[(oe, st)], st, do, wt, i,
                                      start=(do == 0))
                    if half_pass == 1 and pi == 4:
                        # o-block 3 prefetch mid-B (slots free by now)
                        for c3 in range(len(W_CH)):
                            push_w(3, c3)
                        for h3 in range(len(W8_CH)):
                            push_w8(3, h3)
                for h in range(len(W8_CH)):
                    for oe in range(2):
                        wt = w_8[(oe, h)]
                        for st in sts:
                            for lp in range(W8_CH[h] // 2):
                                j = W8_OFF[h] // 2 + lp
                                mm_dr(ps[(oe, st)], st, j, wt, lp,
                                      stop=(j == n_pairs - 1))
                for oe in range(2):
                    for st in sts:
                        evict(oe, st, ps[(oe, st)])

            # ---- o-blocks 2..7: st-major; evictions pipeline.
            for oe in range(2, OE):
                wts = [w_c.pop((oe, c)) for c in range(len(W_CH))]
                w8ts = [w_8.pop((oe, h)) for h in range(len(W8_CH))]
                for st in range(ST):
                    if st == 4 and oe + 2 < OE:
                        # prefetch W two blocks out, mid-block so the
                        # push never waits on a busy slot (no ring stall)
                        for c2 in range(len(W_CH)):
                            push_w(oe + 2, c2)
                        for h2 in range(len(W8_CH)):
                            push_w8(oe + 2, h2)
                    ps_t = pp.tile([P, NB], mybir.dt.float32, tag="ps",
                                   name=f"ps{oe}_{st}")
                    for do in range(BF_DOS):
                        c = min(do // 4, len(W_CH) - 1)
                        mm_bf(ps_t, st, do, wts[c], do - W_OFF[c],
                              start=(do == 0))
                    for j in range(n_pairs):
                        h, lp = h_of[j]
                        mm_dr(ps_t, st, j, w8ts[h], lp,
                              stop=(j == n_pairs - 1))
                    evict(oe, st, ps_t)
    nc.compile()
    return nc


def _get_module():
    if "nc" not in _cache:
        _cache["nc"] = _build_module()
    return _cache["nc"]


def _prep_inputs(x, W, b, A, B):
    """Host-side: fold adapter, transpose to kernel layouts, cast, shard."""
    W_eff = W.astype(np.float32) + SCALE * (
        B.astype(np.float32) @ A.astype(np.float32)
    )
    W16 = W_eff * WSC
    # wT[oe, p, do, oo] = W16[oe*NB+oo, do*P+p]  (= W16^T in [K,N] tiles)
    wfull = np.ascontiguousarray(
        W16.T.reshape(DO, P, OE, NB).transpose(2, 1, 0, 3)
    )
    wT = np.ascontiguousarray(wfull[:, :, :BF_DOS, :]).astype(BF16)
    wT8 = np.ascontiguousarray(wfull[:, :, BF_DOS:, :]).astype(FP8)
    x2 = np.asarray(x, dtype=np.float32).reshape(S_FULL, D)
    in_maps = []
    for c in range(N_CORES):
        xc = x2[c * S:(c + 1) * S]                       # (S, D)
        # xT[st, p, do, s'] = xc[st*P+s', do*P+p]  (contiguous per (st, p))
        xfull = np.ascontiguousarray(
            xc.reshape(ST, P, DO, P).transpose(0, 3, 2, 1)
        )
        xTc = np.ascontiguousarray(xfull[:, :, :BF_DOS, :]).astype(BF16)
        xT8c = np.ascontiguousarray(xfull[:, :, BF_DOS:, :]).astype(FP8)
        in_maps.append({"xT": xTc, "xT8": xT8c, "wT": wT, "wT8": wT8})
    return in_maps


def run(x, W, b, A, B, trace=False, **spmd_kwargs):
    """Run the kernel; returns (full_output, BassKernelResults)."""
    from concourse import bass_utils

    nc = _get_module()
    in_maps = _prep_inputs(x, W, b, A, B)
    res = bass_utils.run_bass_kernel_spmd(
        nc, in_maps, core_ids=list(range(N_CORES)), trace=trace, **spmd_kwargs
    )
    outs = [np.asarray(res.results[c]["out"]) for c in range(N_CORES)]
    full = np.concatenate(outs, axis=0).astype(np.float32)
    full += np.asarray(b, dtype=np.float32)
    return full.reshape(4, 2048, O), res


def kernel(x, W, b, A, B):
    full, _ = run(x, W, b, A, B, trace=False)
    return full
